# revision 24
# baseline (speedup 1.0000x reference)
"""Trainium2 Bass kernel for leave-one-out Nadaraya-Watson regression
(nn_Net_7610682049228, retrieval_knn).

Math
----
Zw = relu(x @ W1.T) @ W2.T          [N, 3]
Xw = relu(train_X @ W1.T) @ W2.T    [N, 3]
K[i,j,d] = exp(-((Xw[j,d]-Zw[i,d])/h)^2 / 2), diagonal i==j masked out
out[i,d] = sum_j K*Y / sum_j K

Kernel factorization (the key trick):
  K[i,j,d] = G[j,d] * H[i,d] * C[i,j,d]
    G[j,d] = exp(-Xw[j,d]^2 / 2h^2)        (O(N) precompute)
    H[i,d] = exp(-Zw[i,d]^2 / 2h^2)        (cancels in the ratio!)
    C[i,j,d] = exp(Zw[i,d]*Xw[j,d] / h^2)  (rank-1 exponent)
  out[i,d] = (sum_j C*G*Y - c_i*Y_i) / (sum_j C*G - c_i)
    with the leave-one-out correction c[i,d] = exp((Zw*Xw - Xw^2/2)/h^2)|_{j=i}.

So the only O(N^2) work is: a rank-1 outer product (DVE tensor_scalar with a
per-partition scalar), one big Exp pass (ACT engine - the throughput floor),
and [G*Y | G]-weighted column reductions (PE matmuls accumulating in PSUM).

Sharding: data-parallel over query rows i; core m handles i in
[512m, 512m+512). j lives on SBUF partitions (32 blocks of 128), the 512
i-columns of the shard live on the free dim. No cross-core communication.

All input-dependent scalars (h, W2) are consumed as tensors, so the compiled
program is input-independent and built/compiled once per process.
"""

import numpy as np
from contextlib import ExitStack

import concourse.bacc as bacc
import concourse.bass as bass
import concourse.mybir as mybir
import concourse.tile as tile
from concourse.bass_utils import run_bass_kernel_spmd

F32 = mybir.dt.float32
AF = mybir.ActivationFunctionType
OP = mybir.AluOpType

N = 4096
NCORES = 8
SHARD = N // NCORES          # 512 query rows per core
P = 128                      # SBUF partitions
JB = N // P                  # 32 j-blocks
D = 3                        # output dims
JB_PER_CHUNK = 4             # j-blocks fused into one ACT Exp instruction
NCHUNK = JB // JB_PER_CHUNK  # 8
CHUNK_W = JB_PER_CHUNK * D * SHARD  # 6144 free elements per chunk

_CACHE = {}


def _build_program() -> bass.Bass:
    # Bacc (not raw Bass): its compile() pass legalizes multi-wait
    # instructions for walrus, which allows only 1-2 sync waits per op.
    nc = bacc.Bacc("TRN2", target_bir_lowering=False, debug=False)

    # --- DRAM I/O (per-core shapes; host preps layouts/slices) ---
    d_xTs = nc.dram_tensor("xTs", (4, SHARD), F32, kind="ExternalInput").ap()
    d_tXTs = nc.dram_tensor("tXTs", (4, SHARD), F32, kind="ExternalInput").ap()
    d_tXT = nc.dram_tensor("tXT", (4, N), F32, kind="ExternalInput").ap()
    d_Yj = nc.dram_tensor("Yj", (P, JB * D), F32, kind="ExternalInput").ap()
    d_YTs = nc.dram_tensor("YTs", (D, SHARD), F32, kind="ExternalInput").ap()
    d_W1T = nc.dram_tensor("W1T", (4, D), F32, kind="ExternalInput").ap()
    d_W2T = nc.dram_tensor("W2T", (D, D), F32, kind="ExternalInput").ap()
    d_W2f = nc.dram_tensor("W2f", (1, D * D), F32, kind="ExternalInput").ap()
    d_h = nc.dram_tensor("hh", (1, 1), F32, kind="ExternalInput").ap()
    d_sel = nc.dram_tensor("sel", (D, D * P), F32, kind="ExternalInput").ap()
    d_outT = nc.dram_tensor("outT", (D, SHARD), F32, kind="ExternalOutput").ap()

    with tile.TileContext(nc) as tc, ExitStack() as ctx:
        sb = ctx.enter_context(tc.tile_pool(name="sb", bufs=1))
        pp = ctx.enter_context(tc.tile_pool(name="pp", bufs=2))
        cp = ctx.enter_context(tc.tile_pool(name="cp", bufs=2))
        ps = ctx.enter_context(tc.tile_pool(name="ps", bufs=1, space="PSUM"))
        pr = ctx.enter_context(tc.tile_pool(name="pr", bufs=1, space="PSUM"))
        # One explicitly reused PSUM scratch tile for all setup matmuls.
        # (A rotating pool would make each new tile's first toucher inherit
        # release-waits from several engines; walrus allows only 2 sync waits
        # per instruction.)
        PS = ps.tile([P, SHARD], F32, tag="scratch", name="PS")

        # ---------- load inputs ----------
        # Tensors consumed by matmuls are staged through a DVE copy: walrus
        # LDWEIGHTS tolerates only ~2 sync waits, and direct DMA-fed matmuls
        # can accumulate waits on several HW DMA queues plus a compute sem.
        # After the copies every matmul dependency is a single DVE semaphore.
        def load_mm(dram_ap, shape, name):
            raw = sb.tile(shape, F32, name=f"{name}_raw")
            nc.gpsimd.dma_start(raw, dram_ap)
            out = sb.tile(shape, F32, name=name)
            nc.vector.tensor_copy(out, raw)
            return out

        xTs = load_mm(d_xTs, [4, SHARD], "xTs")
        tXTs = load_mm(d_tXTs, [4, SHARD], "tXTs")
        tXT = load_mm(d_tXT, [4, N], "tXT")
        W1T = load_mm(d_W1T, [4, D], "W1T")
        W2T = load_mm(d_W2T, [D, D], "W2T")
        # DVE-only consumers can read the DMA'd tile directly.
        W2f = sb.tile([1, D * D], F32)
        nc.gpsimd.dma_start(W2f, d_W2f)
        Yj = sb.tile([P, JB * D], F32)
        nc.gpsimd.dma_start(Yj, d_Yj)
        YTs = sb.tile([D, SHARD], F32)
        nc.gpsimd.dma_start(YTs, d_YTs)
        h_sb = sb.tile([1, 1], F32)
        nc.gpsimd.dma_start(h_sb, d_h)

        ones = sb.tile([1, P], F32)
        nc.vector.memset(ones, 1.0)
        zb = sb.tile([P, 1], F32)  # zero bias for activations
        nc.vector.memset(zb, 0.0)

        # ---------- broadcast scalars: 1/h^2 and W2 across partitions ----------
        hsq = sb.tile([1, 1], F32)
        nc.vector.tensor_mul(hsq, h_sb, h_sb)
        hinv = sb.tile([1, 1], F32)
        nc.vector.reciprocal(hinv, hsq)
        W2h = sb.tile([1, 1 + D * D], F32)  # [1/h^2, W2 row-major]
        nc.vector.tensor_copy(W2h[:, 0:1], hinv)
        nc.vector.tensor_copy(W2h[:, 1:], W2f)
        nc.tensor.matmul(PS[:, 0 : 1 + D * D], ones, W2h, start=True, stop=True)
        bc = sb.tile([P, 1 + D * D], F32)
        nc.vector.tensor_copy(bc, PS[:, 0 : 1 + D * D])
        invh2 = bc[:, 0:1]

        def w2col(d, m):  # W2[d,m] broadcast per-partition
            return bc[:, 1 + D * d + m : 2 + D * d + m]

        nh = sb.tile([P, 1], F32)  # -1/(2 h^2), ACT scale for G
        nc.vector.tensor_scalar_mul(nh, invh2, -0.5)

        # ---------- T-layout MLP: ZwT [3,512] (queries), XwTs [3,512] ----------
        def mlp_T(src, name):
            nc.tensor.matmul(PS[0:D, :], W1T, src, start=True, stop=True)
            hid = sb.tile([D, SHARD], F32, name=f"hid{name}")
            nc.scalar.activation(hid, PS[0:D, :], AF.Relu, bias=zb[0:D, :])
            nc.tensor.matmul(PS[0:D, :], W2T, hid, start=True, stop=True)
            out = sb.tile([D, SHARD], F32, name=f"mlpT{name}")
            nc.vector.tensor_copy(out, PS[0:D, :])
            return out

        ZwT = mlp_T(xTs, "z")      # Zw.T for this core's shard (unscaled)
        XwTs = mlp_T(tXTs, "x")    # Xw.T for the same global rows (unscaled)

        # ---------- j-layout MLP: Xw for all N train rows ----------
        # layer 1 on PE: 32 matmuls [4,128].T @ [4,3] -> one PSUM bank [128,96]
        for jb in range(JB):
            nc.tensor.matmul(
                PS[:, D * jb : D * (jb + 1)],
                tXT[:, P * jb : P * (jb + 1)],
                W1T,
                start=True,
                stop=True,
            )
        h1j = sb.tile([P, JB * D], F32)
        nc.scalar.activation(h1j, PS[:, 0 : JB * D], AF.Relu, bias=zb)
        # layer 2 on DVE with per-partition W2 scalars
        h1r = h1j.rearrange("p (a m) -> p a m", m=D)
        Xwj = sb.tile([P, JB * D], F32)
        Xwr = Xwj.rearrange("p (a d) -> p a d", d=D)
        for d in range(D):
            acc0 = sb.tile([P, JB], F32, tag="l2a", name="acc0")
            nc.vector.tensor_scalar_mul(acc0, h1r[:, :, 0], w2col(d, 0))
            acc1 = sb.tile([P, JB], F32, tag="l2b", name="acc1")
            nc.vector.scalar_tensor_tensor(
                acc1, h1r[:, :, 1], w2col(d, 1), acc0, OP.mult, OP.add
            )
            nc.vector.scalar_tensor_tensor(
                Xwr[:, :, d], h1r[:, :, 2], w2col(d, 2), acc1, OP.mult, OP.add
            )
        # Xw scaled by 1/h^2: the per-partition scalar for the rank-1 products
        Xws = sb.tile([P, JB * D], F32)
        nc.vector.tensor_scalar_mul(Xws, Xwj, invh2)

        # ---------- G, G*Y -> interleaved matmul weights W6 ----------
        sq = sb.tile([P, JB * D], F32)
        nc.vector.tensor_mul(sq, Xwj, Xwj)
        Gj = sb.tile([P, JB * D], F32)
        nc.scalar.activation(Gj, sq, AF.Exp, bias=zb, scale=nh)
        GYj = sb.tile([P, JB * D], F32)
        nc.vector.tensor_mul(GYj, Gj, Yj)
        W6 = sb.tile([P, JB * D * 2], F32)
        W6r = W6.rearrange("p (a t) -> p a t", t=2)
        nc.vector.tensor_copy(W6r[:, :, 0], GYj)
        nc.vector.tensor_copy(W6r[:, :, 1], Gj)

        # ---------- Zw replicated across partitions: [128, 3*512] ----------
        # matmul rhs must start at partition 0, so select row d of ZwT with a
        # one-hot lhsT: Zrep_d = sel_d.T @ ZwT, sel_d[k,p] = (k==d).
        sel = load_mm(d_sel, [D, D * P], "sel")
        Zrep = sb.tile([P, D * SHARD], F32)
        for d in range(D):
            nc.tensor.matmul(
                PS, sel[:, P * d : P * (d + 1)], ZwT, start=True, stop=True
            )
            nc.vector.tensor_copy(Zrep[:, SHARD * d : SHARD * (d + 1)], PS)

        # ---------- main O(N^2) loop ----------
        red = [
            pr.tile([2, SHARD], F32, tag=f"red{d}", name=f"red{d}") for d in range(D)
        ]
        for c in range(NCHUNK):
            Pt = pp.tile([P, CHUNK_W], F32, tag="P", name="Pt")
            Ct = cp.tile([P, CHUNK_W], F32, tag="C", name="Ct")
            # First-touch Ct on the ACT engine: waits are per-engine (no
            # transitivity), so the recycled slot's WAR-vs-PE-readers wait
            # must be paid by an ACT instruction. This cheap copy absorbs it;
            # the big Exp then carries only its DVE data dependency
            # (walrus allows at most 2 sync waits per instruction).
            nc.scalar.copy(Ct[:, 0:1], zb)
            for jl in range(JB_PER_CHUNK):
                jb = JB_PER_CHUNK * c + jl
                for d in range(D):
                    off = (jl * D + d) * SHARD
                    nc.vector.tensor_scalar_mul(
                        Pt[:, off : off + SHARD],
                        Zrep[:, SHARD * d : SHARD * (d + 1)],
                        Xws[:, D * jb + d : D * jb + d + 1],
                    )
            nc.scalar.activation(Ct, Pt, AF.Exp, bias=zb)
            for jl in range(JB_PER_CHUNK):
                jb = JB_PER_CHUNK * c + jl
                for d in range(D):
                    off = (jl * D + d) * SHARD
                    nc.tensor.matmul(
                        red[d],
                        W6[:, 6 * jb + 2 * d : 6 * jb + 2 * d + 2],
                        Ct[:, off : off + SHARD],
                        start=(jb == 0),
                        stop=(jb == JB - 1),
                    )

        # ---------- leave-one-out correction + ratio (T-layout, [3,512]) ----------
        t1 = sb.tile([D, SHARD], F32)
        nc.vector.tensor_mul(t1, ZwT, XwTs)
        nhx = sb.tile([D, SHARD], F32)
        nc.vector.tensor_scalar_mul(nhx, XwTs, -0.5)
        t2 = sb.tile([D, SHARD], F32)
        nc.vector.tensor_mul(t2, nhx, XwTs)
        t3 = sb.tile([D, SHARD], F32)  # Zw*Xw - Xw^2/2
        nc.vector.tensor_add(t3, t2, t1)
        cT = sb.tile([D, SHARD], F32)
        nc.scalar.activation(cT, t3, AF.Exp, bias=zb[0:D, :], scale=invh2[0:D, :])
        cY = sb.tile([D, SHARD], F32)
        nc.vector.tensor_mul(cY, cT, YTs)
        # engine ops can't address partition bases 1/2, so gather the PSUM
        # rows into [3,512] tiles via PSUM->SBUF copies + one SBUF DMA per row
        # (a single DMA per consumer keeps every op at <=2 sync waits).
        S6 = sb.tile([2, D * SHARD], F32)
        for d in range(D):
            nc.vector.tensor_copy(S6[:, SHARD * d : SHARD * (d + 1)], red[d])
        SnT = sb.tile([D, SHARD], F32)
        SdT = sb.tile([D, SHARD], F32)
        nc.gpsimd.dma_start(SnT, S6[0:1, :])
        nc.gpsimd.dma_start(SdT, S6[1:2, :])
        numT = sb.tile([D, SHARD], F32)
        nc.vector.tensor_sub(numT, SnT, cY)
        denT = sb.tile([D, SHARD], F32)
        nc.vector.tensor_sub(denT, SdT, cT)
        rT = sb.tile([D, SHARD], F32)
        nc.vector.reciprocal(rT, denT)
        oT = sb.tile([D, SHARD], F32)
        nc.vector.tensor_mul(oT, numT, rT)
        nc.gpsimd.dma_start(d_outT, oT)

    nc.compile()
    return nc


def _get_program() -> bass.Bass:
    if "nc" not in _CACHE:
        _CACHE["nc"] = _build_program()
    return _CACHE["nc"]


def _in_maps(x, train_X, Y, W1, W2, h):
    Yj = np.ascontiguousarray(
        Y.reshape(JB, P, D).transpose(1, 0, 2).reshape(P, JB * D)
    )
    tXT = np.ascontiguousarray(train_X.T)
    W1T = np.ascontiguousarray(W1.T)
    W2T = np.ascontiguousarray(W2.T)
    W2f = np.ascontiguousarray(W2.reshape(1, D * D))
    hh = np.asarray(h, np.float32).reshape(1, 1)
    sel = np.zeros((D, D * P), np.float32)
    for d in range(D):
        sel[d, P * d : P * (d + 1)] = 1.0
    maps = []
    for m in range(NCORES):
        sl = slice(SHARD * m, SHARD * (m + 1))
        maps.append(
            {
                "xTs": np.ascontiguousarray(x[sl].T),
                "tXTs": np.ascontiguousarray(train_X[sl].T),
                "tXT": tXT,
                "Yj": Yj,
                "YTs": np.ascontiguousarray(Y[sl].T),
                "W1T": W1T,
                "W2T": W2T,
                "W2f": W2f,
                "hh": hh,
                "sel": sel,
            }
        )
    return maps


def kernel(x, train_X, Y, W1, W2, h, **run_kwargs):
    x = np.asarray(x, np.float32)
    train_X = np.asarray(train_X, np.float32)
    Y = np.asarray(Y, np.float32)
    W1 = np.asarray(W1, np.float32)
    W2 = np.asarray(W2, np.float32)
    nc = _get_program()
    maps = _in_maps(x, train_X, Y, W1, W2, h)
    rr = run_bass_kernel_spmd(nc, maps, list(range(NCORES)), **run_kwargs)
    out = np.concatenate(
        [np.asarray(rr.results[m]["outT"]).T for m in range(NCORES)], axis=0
    )
    if run_kwargs:
        kernel.last_results = rr
    return np.ascontiguousarray(out, np.float32)
